# revision 10
# baseline (speedup 1.0000x reference)
"""2-layer GraphConv GNN on 8 trn2 NeuronCores (Bass/Tile).

Strategy (hardcoded for N=100000 nodes, E=1600000 edges, F=128, H=128, O=64):
  - Shard edges by destination node: core c owns dst in [c*12500, (c+1)*12500).
  - Aggregation via PE matmul segment-sum: edges chunked 128 at a time;
    msgs [128 edges, 128 feat] (bf16, gathered via dma_gather) as lhsT,
    one-hot S [128 edges, 128 dst-slots] (built on DVE via iota==dst compare)
    as rhs; accumulate into PSUM [128 feat, 128 dst] per 128-dst group.
  - Gather: dma_gather (int16 idx) with sources split into 4 ranges of 25000
    rows; 4 SWDGE queues in parallel. Edges laid out in slots grouped by
    (supergroup, src-range, dst-group), padded to fixed budgets
    (SPMD-uniform across cores; pad slots get dst=200 -> masked by S).
  - Upload minimization: only the core's own x shard (bf16) is uploaded;
    the full gather table is built on-device via AllGather. idx16 uploaded
    16-wide and replicated to 128 partitions on device. Output downloaded
    bf16 and cast to f32 on host.
  - Inter-layer exchange: AllGather of hr = h @ w_rel2.T (bf16) + on-device
    bf16->f32 expand so L2 gathers 256-B rows.
"""

import numpy as np
import ml_dtypes
from contextlib import ExitStack

N = 100000
F = 128          # input/hidden feature dim
O = 64           # output dim
NC = 8
SHARD = N // NC          # 12500
G = 128                  # dst nodes per psum group
NGROUP = (SHARD + G - 1) // G   # 98 (last group has 84 nodes)
LASTG = SHARD - (NGROUP - 1) * G  # 84
NR = 4                   # src ranges (int16 gather index limit)
RS = N // NR             # 25000
SB = 640                 # slot budget per (group, range); 5 chunks of 128
CHUNKS_PER_SEG = SB // 128  # 5
SG_SIZE = 4              # groups per supergroup (gather call batching)

bf16 = ml_dtypes.bfloat16


def _supergroups():
    sgs = []
    g0 = 0
    while g0 < NGROUP:
        sgs.append(list(range(g0, min(g0 + SG_SIZE, NGROUP))))
        g0 += SG_SIZE
    return sgs


SGS = _supergroups()
NCHUNKS = NGROUP * NR * CHUNKS_PER_SEG  # 1960 chunks per layer
TOTSLOTS = NGROUP * NR * SB             # 250880

# slot base per bucket in (sg, r, g_local) call-major order (static layout)
_SLOTBASE = np.zeros(NGROUP * NR, dtype=np.int64)
_pos = 0
for _sg in SGS:
    for _r in range(NR):
        for _g in _sg:
            _SLOTBASE[_g * NR + _r] = _pos
            _pos += SB


def _prep_core(src, dst_local):
    """Slot layout for one core. Returns idx16 [16, TOTSLOTS//16] (int16,
    per-call 16-wrapped) and dstS [128, NCHUNKS] bf16."""
    g = dst_local // G
    r = src >> 14  # src // 16384? no - see below; replaced by // RS
    r = src // RS
    bucket = g * NR + r
    order = np.argsort(bucket, kind="stable")
    s_o = src[order]
    b_o = bucket[order]
    d_o = dst_local[order]
    cnt = np.bincount(b_o, minlength=NGROUP * NR)
    if cnt.max() > SB:
        raise RuntimeError(f"bucket overflow: {cnt.max()} > {SB}")

    start = np.zeros(NGROUP * NR + 1, dtype=np.int64)
    np.cumsum(cnt, out=start[1:])
    within = np.arange(len(b_o)) - start[b_o]
    slot = _SLOTBASE[b_o] + within

    # pad slots gather row 0 of the range (S row is masked via dst=200).
    idx_val = np.zeros(TOTSLOTS, dtype=np.int16)
    idx_val[slot] = (s_o - (s_o // RS) * RS).astype(np.int16)
    dst_val = np.full(TOTSLOTS, 200, dtype=np.float32)
    dst_val[slot] = (d_o % G).astype(np.float32)

    # per-call 16-wrap: call = (sg, r) covering len(sg)*SB slots
    cols = []
    pos = 0
    for sg in SGS:
        ncall = len(sg) * SB
        for r_ in range(NR):
            blk = idx_val[pos : pos + ncall]
            cols.append(blk.reshape(ncall // 16, 16).T)  # [16, ncall/16]
            pos += ncall
    idx16 = np.concatenate(cols, axis=1)  # [16, TOTSLOTS/16]

    dstS = np.ascontiguousarray(dst_val.reshape(NCHUNKS, 128).T).astype(bf16)
    return idx16, dstS


import os
_L1ONLY = bool(int(os.environ.get("GNN_L1ONLY", "0")))


def input_decls():
    """(name, shape, dtype) for every ExternalInput — shared with bench_null."""
    return [
        ("xs", [SHARD, F], "bfloat16"),
        ("idx16s", [16, TOTSLOTS // 16], "int16"),
        ("dstS", [128, NCHUNKS], "bfloat16"),
        ("wr1T", [F, F], "float32"),
        ("wo1T", [F, F], "float32"),
        ("wr2T", [F, O], "float32"),
        ("wo2T", [F, O], "float32"),
        ("b1", [1, F], "float32"),
        ("b2", [1, O], "float32"),
        ("iota", [128, G], "bfloat16"),
        ("iota32", [128, G], "float32"),
        ("identb", [128, 128], "bfloat16"),
        ("ident", [128, 128], "float32"),
        ("ones", [1, G], "float32"),
    ]


def _build_program():
    import concourse.bass as bass
    import concourse.tile as tile
    from concourse import bacc, mybir

    nc = bacc.Bacc(None, target_bir_lowering=False, num_swdge_queues=4)
    dt = mybir.dt

    # inputs
    xs_in = nc.dram_tensor("xs", [SHARD, F], dt.bfloat16, kind="ExternalInput")
    idx16s = nc.dram_tensor("idx16s", [16, TOTSLOTS // 16], dt.int16, kind="ExternalInput")
    dstS_in = nc.dram_tensor("dstS", [128, NCHUNKS], dt.bfloat16, kind="ExternalInput")
    wr1T = nc.dram_tensor("wr1T", [F, F], dt.float32, kind="ExternalInput")
    wo1T = nc.dram_tensor("wo1T", [F, F], dt.float32, kind="ExternalInput")
    wr2T = nc.dram_tensor("wr2T", [F, O], dt.float32, kind="ExternalInput")
    wo2T = nc.dram_tensor("wo2T", [F, O], dt.float32, kind="ExternalInput")
    b1_in = nc.dram_tensor("b1", [1, F], dt.float32, kind="ExternalInput")
    b2_in = nc.dram_tensor("b2", [1, O], dt.float32, kind="ExternalInput")
    iota_in = nc.dram_tensor("iota", [128, G], dt.bfloat16, kind="ExternalInput")
    iota32_in = nc.dram_tensor("iota32", [128, G], dt.float32, kind="ExternalInput")
    identb_in = nc.dram_tensor("identb", [128, 128], dt.bfloat16, kind="ExternalInput")
    ident_in = nc.dram_tensor("ident", [128, 128], dt.float32, kind="ExternalInput")
    ones_in = nc.dram_tensor("ones", [1, G], dt.float32, kind="ExternalInput")
    out_t = nc.dram_tensor("out", [SHARD, O], dt.bfloat16, kind="ExternalOutput")

    # internal DRAM
    xs_int = nc.dram_tensor("xs_int", [SHARD, F], dt.bfloat16)
    xfull = nc.dram_tensor("xfull", [N, F], dt.bfloat16, addr_space="Shared")
    idx16f = nc.dram_tensor("idx16f", [128, TOTSLOTS // 16], dt.int16)
    hr_shard = nc.dram_tensor("hr_shard", [SHARD, O], dt.bfloat16)
    hr_full_bf = nc.dram_tensor("hr_full_bf", [N, O], dt.bfloat16, addr_space="Shared")
    hr_full = nc.dram_tensor("hr_full", [N, O], dt.float32)

    with tile.TileContext(nc) as tc, ExitStack() as ctx:
        const_p = ctx.enter_context(tc.tile_pool(name="const", bufs=1))
        resid_p = ctx.enter_context(tc.tile_pool(name="resid", bufs=1))
        idx_p = ctx.enter_context(tc.tile_pool(name="idxp", bufs=8))
        msgs_p = ctx.enter_context(tc.tile_pool(name="msgs", bufs=8))
        s_p = ctx.enter_context(tc.tile_pool(name="sp", bufs=8))
        t_p = ctx.enter_context(tc.tile_pool(name="tp", bufs=2))
        agg_p = ctx.enter_context(tc.tile_pool(name="aggp", bufs=3))
        hsb_p = ctx.enter_context(tc.tile_pool(name="hsb", bufs=3))
        osb_p = ctx.enter_context(tc.tile_pool(name="osb", bufs=3))
        xt_p = ctx.enter_context(tc.tile_pool(name="xtp", bufs=3))
        ps_agg = ctx.enter_context(tc.tile_pool(name="ps_agg", bufs=2, space="PSUM"))
        ps_h = ctx.enter_context(tc.tile_pool(name="ps_h", bufs=2, space="PSUM"))
        ps_t = ctx.enter_context(tc.tile_pool(name="ps_t", bufs=2, space="PSUM"))

        # --- prologue: build gather table + idx replication + residents ---
        # xs -> internal -> AllGather to full bf16 table
        nc.sync.dma_start(xs_int[:], xs_in[:])
        nc.gpsimd.collective_compute(
            "AllGather",
            mybir.AluOpType.bypass,
            replica_groups=[list(range(NC))],
            ins=[xs_int[:]],
            outs=[xfull[:]],
        )
        # idx16 [16, C] -> [128, C] on-device replication
        for k in range(8):
            nc.sync.dma_start(idx16f[16 * k : 16 * (k + 1), :], idx16s[:])

        # constants / residents
        c_iota = const_p.tile([128, G], dt.bfloat16)
        nc.sync.dma_start(c_iota[:], iota_in[:])
        c_iota32 = const_p.tile([128, G], dt.float32)
        nc.sync.dma_start(c_iota32[:], iota32_in[:])
        c_identb = const_p.tile([128, 128], dt.bfloat16)
        nc.sync.dma_start(c_identb[:], identb_in[:])
        c_ident = const_p.tile([128, 128], dt.float32)
        nc.sync.dma_start(c_ident[:], ident_in[:])
        c_ones = const_p.tile([1, G], dt.float32)
        nc.sync.dma_start(c_ones[:], ones_in[:])
        c_wr1T = const_p.tile([F, F], dt.float32)
        nc.sync.dma_start(c_wr1T[:], wr1T[:])
        c_wo1T = const_p.tile([F, F], dt.float32)
        nc.sync.dma_start(c_wo1T[:], wo1T[:])
        c_wr2T = const_p.tile([F, O], dt.float32)
        nc.sync.dma_start(c_wr2T[:], wr2T[:])
        c_wo2T = const_p.tile([F, O], dt.float32)
        nc.sync.dma_start(c_wo2T[:], wo2T[:])
        c_b1 = const_p.tile([1, F], dt.float32)
        nc.sync.dma_start(c_b1[:], b1_in[:])
        c_b2 = const_p.tile([1, O], dt.float32)
        nc.sync.dma_start(c_b2[:], b2_in[:])
        c_dstS = const_p.tile([128, NCHUNKS], dt.bfloat16)
        nc.sync.dma_start(c_dstS[:], dstS_in[:])
        c_dstS32 = const_p.tile([128, NCHUNKS], dt.float32)
        nc.scalar.copy(out=c_dstS32[:], in_=c_dstS[:])

        # r_xiT [F, SHARD] f32: transpose the bf16 shard on PE
        r_xiT = resid_p.tile([F, SHARD], dt.float32)
        for g_ in range(NGROUP):
            ngn = G if g_ < NGROUP - 1 else LASTG
            gbase = g_ * G
            xt = xt_p.tile([128, F], dt.bfloat16, tag="xt")
            nc.sync.dma_start(xt[:ngn, :], xs_in[gbase : gbase + ngn, :])
            pt = ps_t.tile([128, 128], dt.bfloat16, tag="pt", space="PSUM")
            nc.tensor.transpose(pt[:F, :ngn], xt[:ngn, :F], c_identb[:ngn, :ngn])
            nc.scalar.copy(out=r_xiT[:, gbase : gbase + ngn], in_=pt[:F, :ngn])
        r_hT = resid_p.tile([F, SHARD], dt.float32)  # written in L1, read in L2

        def layer(L):
            """L=1: table=xfull, produce h (hT resident + hr_shard DRAM).
            L=2: table=hr_full, produce out."""
            table = xfull if L == 1 else hr_full
            call_idx = 0   # column offset into idx16f (units of 16-wrapped cols)
            chunk_idx = 0  # global chunk counter (dstS column)
            for sg in SGS:
                ng = len(sg)
                call_slots = ng * SB
                call_cols = call_slots // 16
                blocks = call_slots // 128
                msgs = []
                for r_ in range(NR):
                    it = idx_p.tile([128, call_cols], dt.int16, tag="idx")
                    nc.sync.dma_start(
                        it[:], idx16f[:, call_idx : call_idx + call_cols]
                    )
                    FW = F if L == 1 else O
                    mdt = dt.bfloat16 if L == 1 else dt.float32
                    m = msgs_p.tile([128, blocks * FW], mdt, tag="m" + str(L))
                    nc.gpsimd.dma_gather(
                        m[:].rearrange("p (c e) -> p c e", e=FW),
                        table[r_ * RS : (r_ + 1) * RS, :],
                        it[:],
                        call_slots,
                        call_slots,
                        FW,
                        single_packet=False,
                        queue_num=r_,
                    )
                    msgs.append(m)
                    call_idx += call_cols
                for gl, g_ in enumerate(sg):
                    ngn = G if g_ < NGROUP - 1 else LASTG
                    gbase = g_ * G
                    psum = ps_agg.tile([128, G], dt.float32, tag="agg", space="PSUM")
                    nmm = NR * CHUNKS_PER_SEG
                    mm = 0
                    for r_ in range(NR):
                        for k in range(CHUNKS_PER_SEG):
                            b = gl * CHUNKS_PER_SEG + k
                            # chunk index in slot layout: (sg, r, g_local, k)
                            ci = chunk_idx + (r_ * ng + gl) * CHUNKS_PER_SEG + k
                            sdt = dt.bfloat16 if L == 1 else dt.float32
                            # one-hot S on ACT engine (keeps DVE off the
                            # critical path): S = relu(1 - |dst - iota|)
                            tt = t_p.tile([128, G], dt.float32, tag="tt")
                            nc.scalar.activation(
                                out=tt[:],
                                in_=c_iota32[:],
                                func=mybir.ActivationFunctionType.Abs,
                                bias=c_dstS32[:, ci : ci + 1],
                                scale=-1.0,
                            )
                            S = s_p.tile([128, G], sdt, tag="S" + str(L))
                            nc.scalar.activation(
                                out=S[:],
                                in_=tt[:],
                                func=mybir.ActivationFunctionType.Relu,
                                bias=1.0,
                                scale=-1.0,
                            )
                            FW = F if L == 1 else O
                            nc.tensor.matmul(
                                psum[:FW, :],
                                lhsT=msgs[r_][:, b * FW : (b + 1) * FW],
                                rhs=S[:],
                                start=(mm == 0),
                                stop=(mm == nmm - 1),
                            )
                            mm += 1
                    FW = F if L == 1 else O
                    aggT = agg_p.tile([128, G], dt.float32, tag="aggT")
                    nc.scalar.copy(out=aggT[:FW, :], in_=psum[:FW, :])
                    if L == 1:
                        ph = ps_h.tile([128, G], dt.float32, tag="ph", space="PSUM")
                        nc.tensor.matmul(ph[:], lhsT=c_wr1T[:], rhs=aggT[:], start=True, stop=False)
                        nc.tensor.matmul(ph[:, :ngn], lhsT=c_wo1T[:], rhs=r_xiT[:, gbase : gbase + ngn], start=False, stop=False)
                        nc.tensor.matmul(ph[:, :ngn], lhsT=c_b1[:1, :], rhs=c_ones[:1, :ngn], start=False, stop=True)
                        # relu -> hT resident (fp32)
                        nc.scalar.activation(
                            out=r_hT[:, gbase : gbase + ngn],
                            in_=ph[:, :ngn],
                            func=mybir.ActivationFunctionType.Relu,
                        )
                        # hrT = w_rel2.T-transform of hT slice (feature-major)
                        phr = ps_t.tile([128, 128], dt.float32, tag="pt", space="PSUM")
                        nc.tensor.matmul(phr[:O, :ngn], lhsT=c_wr2T[:], rhs=r_hT[:, gbase : gbase + ngn], start=True, stop=True)
                        hrT = hsb_p.tile([128, G], dt.float32, tag="hrT")
                        nc.scalar.copy(out=hrT[:O, :ngn], in_=phr[:O, :ngn])
                        # transpose -> node-major hr (bf16) -> DRAM for AllGather
                        pt = ps_t.tile([128, 128], dt.float32, tag="pt", space="PSUM")
                        nc.tensor.transpose(pt[:ngn, :O], hrT[:O, :ngn], c_ident[:O, :O])
                        hsb = hsb_p.tile([128, O], dt.bfloat16, tag="hsb")
                        nc.scalar.copy(out=hsb[:ngn, :], in_=pt[:ngn, :O])
                        nc.sync.dma_start(hr_shard[gbase : gbase + ngn, :], hsb[:ngn, :])
                    else:
                        po = ps_h.tile([128, O], dt.float32, tag="po", space="PSUM")
                        # agg2 already rel2-transformed: just transpose to node-major
                        nc.tensor.matmul(po[:ngn, :], lhsT=aggT[:O, :ngn], rhs=c_ident[:O, :O], start=True, stop=False, is_transpose=True)
                        nc.tensor.matmul(po[:ngn, :], lhsT=r_hT[:, gbase : gbase + ngn], rhs=c_wo2T[:], start=False, stop=False)
                        nc.tensor.matmul(po[:ngn, :], lhsT=c_ones[:1, :ngn], rhs=c_b2[:1, :], start=False, stop=True)
                        osb = osb_p.tile([128, O], dt.bfloat16, tag="osb")
                        nc.scalar.copy(out=osb[:ngn, :], in_=po[:ngn, :])
                        nc.sync.dma_start(out_t[gbase : gbase + ngn, :], osb[:ngn, :])
                chunk_idx += ng * NR * CHUNKS_PER_SEG

        layer(1)
        if _L1ONLY:
            zo = osb_p.tile([128, O], dt.bfloat16, tag="osb")
            nc.vector.memset(zo[:], 0.0)
            nc.sync.dma_start(out_t[0:128, :], zo[:])
        else:
            nc.gpsimd.collective_compute(
                "AllGather",
                mybir.AluOpType.bypass,
                replica_groups=[list(range(NC))],
                ins=[hr_shard[:]],
                outs=[hr_full_bf[:]],
            )
            # expand bf16 -> fp32 (cast-DMA, DRAM->DRAM) so L2 gathers 256-B rows
            flat_bf = hr_full_bf[:].rearrange("n o -> (n o)").rearrange("(a b) -> a b", a=128)
            flat_f32 = hr_full[:].rearrange("n o -> (n o)").rearrange("(a b) -> a b", a=128)
            CW = flat_bf.shape[1]
            step = CW // 10
            for i in range(10):
                lo, hi = i * step, (i + 1) * step if i < 9 else CW
                nc.gpsimd.dma_start(flat_f32[:, lo:hi], flat_bf[:, lo:hi])
            layer(2)

    nc.finalize()
    return nc


_CACHED = {}


def prepare_in_maps(inputs):
    x = np.asarray(inputs["x"], dtype=np.float32)
    edge_index = np.asarray(inputs["edge_index"])
    w_rel1 = np.asarray(inputs["w_rel1"], dtype=np.float32)
    b_rel1 = np.asarray(inputs["b_rel1"], dtype=np.float32)
    w_root1 = np.asarray(inputs["w_root1"], dtype=np.float32)
    w_rel2 = np.asarray(inputs["w_rel2"], dtype=np.float32)
    b_rel2 = np.asarray(inputs["b_rel2"], dtype=np.float32)
    w_root2 = np.asarray(inputs["w_root2"], dtype=np.float32)

    src = edge_index[0].astype(np.int64)
    dst = edge_index[1].astype(np.int64)

    xbf = x.astype(bf16)
    iota = np.broadcast_to(np.arange(G, dtype=np.float32), (128, G)).astype(bf16)
    identb = np.eye(128, dtype=np.float32).astype(bf16)
    ident = np.eye(128, dtype=np.float32)
    ones = np.ones((1, G), dtype=np.float32)
    iota32 = np.broadcast_to(np.arange(G, dtype=np.float32), (128, G)).copy()

    # sort all edges once by core, then per-core prep
    core = dst // SHARD
    order = np.argsort(core, kind="stable")
    src_s, dst_s = src[order], dst[order]
    bounds = np.searchsorted(core[order], np.arange(NC + 1))

    in_maps = []
    for c in range(NC):
        lo, hi = bounds[c], bounds[c + 1]
        idx16, dstS = _prep_core(src_s[lo:hi], dst_s[lo:hi] - c * SHARD)
        in_maps.append(
            {
                "xs": xbf[c * SHARD : (c + 1) * SHARD, :],
                "idx16s": idx16,
                "dstS": dstS,
                "wr1T": np.ascontiguousarray(w_rel1.T),
                "wo1T": np.ascontiguousarray(w_root1.T),
                "wr2T": np.ascontiguousarray(w_rel2.T),
                "wo2T": np.ascontiguousarray(w_root2.T),
                "b1": b_rel1.reshape(1, F),
                "b2": b_rel2.reshape(1, O),
                "iota": iota,
                "iota32": iota32,
                "identb": identb,
                "ident": ident,
                "ones": ones,
            }
        )
    return in_maps


def get_nc():
    if "nc" not in _CACHED:
        _CACHED["nc"] = _build_program()
    return _CACHED["nc"]


def kernel(**inputs):
    from concourse.bass_utils import run_bass_kernel_spmd

    in_maps = prepare_in_maps(inputs)
    nc = get_nc()
    res = run_bass_kernel_spmd(nc, in_maps, core_ids=list(range(NC)), trace=False)
    out = np.concatenate([res.results[c]["out"] for c in range(NC)], axis=0)
    return out.astype(np.float32)


# revision 11
# speedup vs baseline: 5.7130x; 5.7130x over previous
"""2-layer GraphConv GNN on 8 trn2 NeuronCores (Bass/Tile).

Strategy (hardcoded for N=100000 nodes, E=1600000 edges, F=128, H=128, O=64):
  - Shard edges by destination node: core c owns dst in [c*12500, (c+1)*12500).
  - Aggregation via PE matmul segment-sum: edges chunked 128 at a time;
    msgs [128 edges, 128 feat] (bf16, gathered via dma_gather) as lhsT,
    one-hot S [128 edges, 128 dst-slots] (built on DVE via iota==dst compare)
    as rhs; accumulate into PSUM [128 feat, 128 dst] per 128-dst group.
  - Gather: dma_gather (int16 idx) with sources split into 4 ranges of 25000
    rows; 4 SWDGE queues in parallel. Edges laid out in slots grouped by
    (supergroup, src-range, dst-group), padded to fixed budgets
    (SPMD-uniform across cores; pad slots get dst=200 -> masked by S).
  - Upload minimization: only the core's own x shard (bf16) is uploaded;
    the full gather table is built on-device via AllGather. idx16 uploaded
    16-wide and replicated to 128 partitions on device. Output downloaded
    bf16 and cast to f32 on host.
  - Inter-layer exchange: AllGather of hr = h @ w_rel2.T (bf16) + on-device
    bf16->f32 expand so L2 gathers 256-B rows.
"""

import numpy as np
import ml_dtypes
from contextlib import ExitStack

N = 100000
F = 128          # input/hidden feature dim
O = 64           # output dim
NC = 8
SHARD = N // NC          # 12500
G = 128                  # dst nodes per psum group
NGROUP = (SHARD + G - 1) // G   # 98 (last group has 84 nodes)
LASTG = SHARD - (NGROUP - 1) * G  # 84
NR = 4                   # src ranges (int16 gather index limit)
RS = N // NR             # 25000
SB = 640                 # slot budget per (group, range); 5 chunks of 128
CHUNKS_PER_SEG = SB // 128  # 5
SG_SIZE = 4              # groups per supergroup (gather call batching)

bf16 = ml_dtypes.bfloat16


def _supergroups():
    sgs = []
    g0 = 0
    while g0 < NGROUP:
        sgs.append(list(range(g0, min(g0 + SG_SIZE, NGROUP))))
        g0 += SG_SIZE
    return sgs


SGS = _supergroups()
NCHUNKS = NGROUP * NR * CHUNKS_PER_SEG  # 1960 chunks per layer
TOTSLOTS = NGROUP * NR * SB             # 250880

# slot base per bucket in (sg, r, g_local) call-major order (static layout)
_SLOTBASE = np.zeros(NGROUP * NR, dtype=np.int64)
_pos = 0
for _sg in SGS:
    for _r in range(NR):
        for _g in _sg:
            _SLOTBASE[_g * NR + _r] = _pos
            _pos += SB


def _prep_core(src, dst_local):
    """Slot layout for one core. Returns idx16 [16, TOTSLOTS//16] (int16,
    per-call 16-wrapped) and dstS [128, NCHUNKS] bf16."""
    g = dst_local // G
    r = src >> 14  # src // 16384? no - see below; replaced by // RS
    r = src // RS
    bucket = g * NR + r
    order = np.argsort(bucket, kind="stable")
    s_o = src[order]
    b_o = bucket[order]
    d_o = dst_local[order]
    cnt = np.bincount(b_o, minlength=NGROUP * NR)
    if cnt.max() > SB:
        raise RuntimeError(f"bucket overflow: {cnt.max()} > {SB}")

    start = np.zeros(NGROUP * NR + 1, dtype=np.int64)
    np.cumsum(cnt, out=start[1:])
    within = np.arange(len(b_o)) - start[b_o]
    slot = _SLOTBASE[b_o] + within

    # pad slots gather row 0 of the range (S row is masked via dst=200).
    idx_val = np.zeros(TOTSLOTS, dtype=np.int16)
    idx_val[slot] = (s_o - (s_o // RS) * RS).astype(np.int16)
    dst_val = np.full(TOTSLOTS, 200, dtype=np.float32)
    dst_val[slot] = (d_o % G).astype(np.float32)

    # per-call 16-wrap: call = (sg, r) covering len(sg)*SB slots
    cols = []
    pos = 0
    for sg in SGS:
        ncall = len(sg) * SB
        for r_ in range(NR):
            blk = idx_val[pos : pos + ncall]
            cols.append(blk.reshape(ncall // 16, 16).T)  # [16, ncall/16]
            pos += ncall
    idx16 = np.concatenate(cols, axis=1)  # [16, TOTSLOTS/16]

    dstS = np.ascontiguousarray(dst_val.reshape(NCHUNKS, 128).T).astype(bf16)
    return idx16, dstS


import os
_L1ONLY = bool(int(os.environ.get("GNN_L1ONLY", "0")))


def input_decls():
    """(name, shape, dtype) for every ExternalInput — shared with bench_null."""
    return [
        ("xs", [SHARD, F], "bfloat16"),
        ("idx16s", [16, TOTSLOTS // 16], "int16"),
        ("dstS", [128, NCHUNKS], "bfloat16"),
        ("wr1T", [F, F], "float32"),
        ("wo1T", [F, F], "float32"),
        ("wr2T", [F, O], "float32"),
        ("wo2T", [F, O], "float32"),
        ("b1", [1, F], "float32"),
        ("b2", [1, O], "float32"),
        ("iota", [128, G], "bfloat16"),
        ("iota32", [128, G], "float32"),
        ("identb", [128, 128], "bfloat16"),
        ("ident", [128, 128], "float32"),
        ("ones", [1, G], "float32"),
    ]


def _build_program():
    import concourse.bass as bass
    import concourse.tile as tile
    from concourse import bacc, mybir

    nc = bacc.Bacc(None, target_bir_lowering=False, num_swdge_queues=4)
    dt = mybir.dt

    # inputs
    xs_in = nc.dram_tensor("xs", [SHARD, F], dt.bfloat16, kind="ExternalInput")
    idx16s = nc.dram_tensor("idx16s", [16, TOTSLOTS // 16], dt.int16, kind="ExternalInput")
    dstS_in = nc.dram_tensor("dstS", [128, NCHUNKS], dt.bfloat16, kind="ExternalInput")
    wr1T = nc.dram_tensor("wr1T", [F, F], dt.float32, kind="ExternalInput")
    wo1T = nc.dram_tensor("wo1T", [F, F], dt.float32, kind="ExternalInput")
    wr2T = nc.dram_tensor("wr2T", [F, O], dt.float32, kind="ExternalInput")
    wo2T = nc.dram_tensor("wo2T", [F, O], dt.float32, kind="ExternalInput")
    b1_in = nc.dram_tensor("b1", [1, F], dt.float32, kind="ExternalInput")
    b2_in = nc.dram_tensor("b2", [1, O], dt.float32, kind="ExternalInput")
    iota_in = nc.dram_tensor("iota", [128, G], dt.bfloat16, kind="ExternalInput")
    iota32_in = nc.dram_tensor("iota32", [128, G], dt.float32, kind="ExternalInput")
    identb_in = nc.dram_tensor("identb", [128, 128], dt.bfloat16, kind="ExternalInput")
    ident_in = nc.dram_tensor("ident", [128, 128], dt.float32, kind="ExternalInput")
    ones_in = nc.dram_tensor("ones", [1, G], dt.float32, kind="ExternalInput")
    out_t = nc.dram_tensor("out", [SHARD, O], dt.bfloat16, kind="ExternalOutput")

    # internal DRAM
    xs_int = nc.dram_tensor("xs_int", [SHARD, F], dt.bfloat16)
    xfull = nc.dram_tensor("xfull", [N, F], dt.bfloat16, addr_space="Shared")
    idx16f = nc.dram_tensor("idx16f", [128, TOTSLOTS // 16], dt.int16)
    hr_shard = nc.dram_tensor("hr_shard", [SHARD, O], dt.bfloat16)
    hr_full_bf = nc.dram_tensor("hr_full_bf", [N, O], dt.bfloat16, addr_space="Shared")
    hr_full = nc.dram_tensor("hr_full", [N, O], dt.float32)

    with tile.TileContext(nc) as tc, ExitStack() as ctx:
        const_p = ctx.enter_context(tc.tile_pool(name="const", bufs=1))
        resid_p = ctx.enter_context(tc.tile_pool(name="resid", bufs=1))
        idx_p = ctx.enter_context(tc.tile_pool(name="idxp", bufs=8))
        msgs_p = ctx.enter_context(tc.tile_pool(name="msgs", bufs=8))
        s_p = ctx.enter_context(tc.tile_pool(name="sp", bufs=8))
        agg_p = ctx.enter_context(tc.tile_pool(name="aggp", bufs=3))
        hsb_p = ctx.enter_context(tc.tile_pool(name="hsb", bufs=3))
        osb_p = ctx.enter_context(tc.tile_pool(name="osb", bufs=3))
        xt_p = ctx.enter_context(tc.tile_pool(name="xtp", bufs=3))
        ps_agg = ctx.enter_context(tc.tile_pool(name="ps_agg", bufs=2, space="PSUM"))
        ps_h = ctx.enter_context(tc.tile_pool(name="ps_h", bufs=2, space="PSUM"))
        ps_t = ctx.enter_context(tc.tile_pool(name="ps_t", bufs=2, space="PSUM"))

        # --- prologue: build gather table + idx replication + residents ---
        # xs -> internal -> AllGather to full bf16 table
        nc.sync.dma_start(xs_int[:], xs_in[:])
        nc.gpsimd.collective_compute(
            "AllGather",
            mybir.AluOpType.bypass,
            replica_groups=[list(range(NC))],
            ins=[xs_int[:]],
            outs=[xfull[:]],
        )
        # idx16 [16, C] -> [128, C] on-device replication
        for k in range(8):
            nc.sync.dma_start(idx16f[16 * k : 16 * (k + 1), :], idx16s[:])

        # constants / residents
        c_iota = const_p.tile([128, G], dt.bfloat16)
        nc.sync.dma_start(c_iota[:], iota_in[:])
        c_iota32 = const_p.tile([128, G], dt.float32)
        nc.sync.dma_start(c_iota32[:], iota32_in[:])
        c_identb = const_p.tile([128, 128], dt.bfloat16)
        nc.sync.dma_start(c_identb[:], identb_in[:])
        c_ident = const_p.tile([128, 128], dt.float32)
        nc.sync.dma_start(c_ident[:], ident_in[:])
        c_ones = const_p.tile([1, G], dt.float32)
        nc.sync.dma_start(c_ones[:], ones_in[:])
        c_wr1T = const_p.tile([F, F], dt.float32)
        nc.sync.dma_start(c_wr1T[:], wr1T[:])
        c_wo1T = const_p.tile([F, F], dt.float32)
        nc.sync.dma_start(c_wo1T[:], wo1T[:])
        c_wr2T = const_p.tile([F, O], dt.float32)
        nc.sync.dma_start(c_wr2T[:], wr2T[:])
        c_wo2T = const_p.tile([F, O], dt.float32)
        nc.sync.dma_start(c_wo2T[:], wo2T[:])
        c_b1 = const_p.tile([1, F], dt.float32)
        nc.sync.dma_start(c_b1[:], b1_in[:])
        c_b2 = const_p.tile([1, O], dt.float32)
        nc.sync.dma_start(c_b2[:], b2_in[:])
        c_dstS = const_p.tile([128, NCHUNKS], dt.bfloat16)
        nc.sync.dma_start(c_dstS[:], dstS_in[:])
        c_dstS32 = const_p.tile([128, NCHUNKS], dt.float32)
        nc.scalar.copy(out=c_dstS32[:], in_=c_dstS[:])

        # r_xiT [F, SHARD] f32: transpose the bf16 shard on PE
        r_xiT = resid_p.tile([F, SHARD], dt.float32)
        for g_ in range(NGROUP):
            ngn = G if g_ < NGROUP - 1 else LASTG
            gbase = g_ * G
            xt = xt_p.tile([128, F], dt.bfloat16, tag="xt")
            nc.sync.dma_start(xt[:ngn, :], xs_in[gbase : gbase + ngn, :])
            pt = ps_t.tile([128, 128], dt.bfloat16, tag="pt", space="PSUM")
            nc.tensor.transpose(pt[:F, :ngn], xt[:ngn, :F], c_identb[:ngn, :ngn])
            nc.scalar.copy(out=r_xiT[:, gbase : gbase + ngn], in_=pt[:F, :ngn])
        r_hT = resid_p.tile([F, SHARD], dt.float32)  # written in L1, read in L2

        def layer(L):
            """L=1: table=xfull, produce h (hT resident + hr_shard DRAM).
            L=2: table=hr_full, produce out."""
            table = xfull if L == 1 else hr_full
            call_idx = 0   # column offset into idx16f (units of 16-wrapped cols)
            chunk_idx = 0  # global chunk counter (dstS column)
            for sg in SGS:
                ng = len(sg)
                call_slots = ng * SB
                call_cols = call_slots // 16
                blocks = call_slots // 128
                msgs = []
                for r_ in range(NR):
                    it = idx_p.tile([128, call_cols], dt.int16, tag="idx")
                    nc.sync.dma_start(
                        it[:], idx16f[:, call_idx : call_idx + call_cols]
                    )
                    FW = F if L == 1 else O
                    mdt = dt.bfloat16 if L == 1 else dt.float32
                    m = msgs_p.tile([128, blocks * FW], mdt, tag="m" + str(L))
                    nc.gpsimd.dma_gather(
                        m[:].rearrange("p (c e) -> p c e", e=FW),
                        table[r_ * RS : (r_ + 1) * RS, :],
                        it[:],
                        call_slots,
                        call_slots,
                        FW,
                        single_packet=False,
                        queue_num=r_,
                    )
                    msgs.append(m)
                    call_idx += call_cols
                for gl, g_ in enumerate(sg):
                    ngn = G if g_ < NGROUP - 1 else LASTG
                    gbase = g_ * G
                    psum = ps_agg.tile([128, G], dt.float32, tag="agg", space="PSUM")
                    nmm = NR * CHUNKS_PER_SEG
                    mm = 0
                    for r_ in range(NR):
                        for k in range(CHUNKS_PER_SEG):
                            b = gl * CHUNKS_PER_SEG + k
                            # chunk index in slot layout: (sg, r, g_local, k)
                            ci = chunk_idx + (r_ * ng + gl) * CHUNKS_PER_SEG + k
                            sdt = dt.bfloat16 if L == 1 else dt.float32
                            S = s_p.tile([128, G], sdt, tag="S" + str(L))
                            nc.vector.tensor_scalar(
                                out=S[:],
                                in0=c_iota[:] if L == 1 else c_iota32[:],
                                scalar1=c_dstS32[:, ci : ci + 1],
                                scalar2=None,
                                op0=mybir.AluOpType.is_equal,
                            )
                            FW = F if L == 1 else O
                            nc.tensor.matmul(
                                psum[:FW, :],
                                lhsT=msgs[r_][:, b * FW : (b + 1) * FW],
                                rhs=S[:],
                                start=(mm == 0),
                                stop=(mm == nmm - 1),
                            )
                            mm += 1
                    FW = F if L == 1 else O
                    aggT = agg_p.tile([128, G], dt.float32, tag="aggT")
                    nc.scalar.copy(out=aggT[:FW, :], in_=psum[:FW, :])
                    if L == 1:
                        ph = ps_h.tile([128, G], dt.float32, tag="ph", space="PSUM")
                        nc.tensor.matmul(ph[:], lhsT=c_wr1T[:], rhs=aggT[:], start=True, stop=False)
                        nc.tensor.matmul(ph[:, :ngn], lhsT=c_wo1T[:], rhs=r_xiT[:, gbase : gbase + ngn], start=False, stop=False)
                        nc.tensor.matmul(ph[:, :ngn], lhsT=c_b1[:1, :], rhs=c_ones[:1, :ngn], start=False, stop=True)
                        # relu -> hT resident (fp32)
                        nc.scalar.activation(
                            out=r_hT[:, gbase : gbase + ngn],
                            in_=ph[:, :ngn],
                            func=mybir.ActivationFunctionType.Relu,
                        )
                        # hrT = w_rel2.T-transform of hT slice (feature-major)
                        phr = ps_t.tile([128, 128], dt.float32, tag="pt", space="PSUM")
                        nc.tensor.matmul(phr[:O, :ngn], lhsT=c_wr2T[:], rhs=r_hT[:, gbase : gbase + ngn], start=True, stop=True)
                        hrT = hsb_p.tile([128, G], dt.float32, tag="hrT")
                        nc.scalar.copy(out=hrT[:O, :ngn], in_=phr[:O, :ngn])
                        # transpose -> node-major hr (bf16) -> DRAM for AllGather
                        pt = ps_t.tile([128, 128], dt.float32, tag="pt", space="PSUM")
                        nc.tensor.transpose(pt[:ngn, :O], hrT[:O, :ngn], c_ident[:O, :O])
                        hsb = hsb_p.tile([128, O], dt.bfloat16, tag="hsb")
                        nc.scalar.copy(out=hsb[:ngn, :], in_=pt[:ngn, :O])
                        nc.sync.dma_start(hr_shard[gbase : gbase + ngn, :], hsb[:ngn, :])
                    else:
                        po = ps_h.tile([128, O], dt.float32, tag="po", space="PSUM")
                        # agg2 already rel2-transformed: just transpose to node-major
                        nc.tensor.matmul(po[:ngn, :], lhsT=aggT[:O, :ngn], rhs=c_ident[:O, :O], start=True, stop=False, is_transpose=True)
                        nc.tensor.matmul(po[:ngn, :], lhsT=r_hT[:, gbase : gbase + ngn], rhs=c_wo2T[:], start=False, stop=False)
                        nc.tensor.matmul(po[:ngn, :], lhsT=c_ones[:1, :ngn], rhs=c_b2[:1, :], start=False, stop=True)
                        osb = osb_p.tile([128, O], dt.bfloat16, tag="osb")
                        nc.scalar.copy(out=osb[:ngn, :], in_=po[:ngn, :])
                        nc.sync.dma_start(out_t[gbase : gbase + ngn, :], osb[:ngn, :])
                chunk_idx += ng * NR * CHUNKS_PER_SEG

        layer(1)
        if _L1ONLY:
            zo = osb_p.tile([128, O], dt.bfloat16, tag="osb")
            nc.vector.memset(zo[:], 0.0)
            nc.sync.dma_start(out_t[0:128, :], zo[:])
        else:
            nc.gpsimd.collective_compute(
                "AllGather",
                mybir.AluOpType.bypass,
                replica_groups=[list(range(NC))],
                ins=[hr_shard[:]],
                outs=[hr_full_bf[:]],
            )
            # expand bf16 -> fp32 (cast-DMA, DRAM->DRAM) so L2 gathers 256-B rows
            flat_bf = hr_full_bf[:].rearrange("n o -> (n o)").rearrange("(a b) -> a b", a=128)
            flat_f32 = hr_full[:].rearrange("n o -> (n o)").rearrange("(a b) -> a b", a=128)
            CW = flat_bf.shape[1]
            step = CW // 10
            for i in range(10):
                lo, hi = i * step, (i + 1) * step if i < 9 else CW
                nc.gpsimd.dma_start(flat_f32[:, lo:hi], flat_bf[:, lo:hi])
            layer(2)

    nc.finalize()
    return nc


_CACHED = {}


def prepare_in_maps(inputs):
    x = np.asarray(inputs["x"], dtype=np.float32)
    edge_index = np.asarray(inputs["edge_index"])
    w_rel1 = np.asarray(inputs["w_rel1"], dtype=np.float32)
    b_rel1 = np.asarray(inputs["b_rel1"], dtype=np.float32)
    w_root1 = np.asarray(inputs["w_root1"], dtype=np.float32)
    w_rel2 = np.asarray(inputs["w_rel2"], dtype=np.float32)
    b_rel2 = np.asarray(inputs["b_rel2"], dtype=np.float32)
    w_root2 = np.asarray(inputs["w_root2"], dtype=np.float32)

    src = edge_index[0].astype(np.int64)
    dst = edge_index[1].astype(np.int64)

    xbf = x.astype(bf16)
    iota = np.broadcast_to(np.arange(G, dtype=np.float32), (128, G)).astype(bf16)
    identb = np.eye(128, dtype=np.float32).astype(bf16)
    ident = np.eye(128, dtype=np.float32)
    ones = np.ones((1, G), dtype=np.float32)
    iota32 = np.broadcast_to(np.arange(G, dtype=np.float32), (128, G)).copy()

    # sort all edges once by core, then per-core prep
    core = dst // SHARD
    order = np.argsort(core, kind="stable")
    src_s, dst_s = src[order], dst[order]
    bounds = np.searchsorted(core[order], np.arange(NC + 1))

    in_maps = []
    for c in range(NC):
        lo, hi = bounds[c], bounds[c + 1]
        idx16, dstS = _prep_core(src_s[lo:hi], dst_s[lo:hi] - c * SHARD)
        in_maps.append(
            {
                "xs": xbf[c * SHARD : (c + 1) * SHARD, :],
                "idx16s": idx16,
                "dstS": dstS,
                "wr1T": np.ascontiguousarray(w_rel1.T),
                "wo1T": np.ascontiguousarray(w_root1.T),
                "wr2T": np.ascontiguousarray(w_rel2.T),
                "wo2T": np.ascontiguousarray(w_root2.T),
                "b1": b_rel1.reshape(1, F),
                "b2": b_rel2.reshape(1, O),
                "iota": iota,
                "iota32": iota32,
                "identb": identb,
                "ident": ident,
                "ones": ones,
            }
        )
    return in_maps


def get_nc():
    if "nc" not in _CACHED:
        _CACHED["nc"] = _build_program()
    return _CACHED["nc"]


def kernel(**inputs):
    from concourse.bass_utils import run_bass_kernel_spmd

    in_maps = prepare_in_maps(inputs)
    nc = get_nc()
    res = run_bass_kernel_spmd(nc, in_maps, core_ids=list(range(NC)), trace=False)
    out = np.concatenate([res.results[c]["out"] for c in range(NC)], axis=0)
    return out.astype(np.float32)
